# revision 22
# baseline (speedup 1.0000x reference)
"""Grouped SwiGLU MoE (M=8192, K=2048, N=1024, E=16, top-2) on 8 TRN2 cores.

Strategy: expert-parallel. Host sorts the M*top_k expanded token slots by
expert, gathers+transposes the activations per core (core c owns the c-th
smallest expert in segment 0 and the c-th largest in segment 1; segments
are zero-padded to static capacities rounded to 8, not 128), and
pre-transposes the three weight tensors to bf16. Each core runs a dense
per-expert GEMM chain:

    H^T[n, m] = silu(Wg^T-contract) * (Wu^T-contract)     (PSUM f32, bf16 out)
    outT[k, m] = Wd^T-stationary-contract over n          (bf16 out, [K, CT])

G3 is transposed (stationary = Wd block, moving = H^T), so its PE cost
scales with actual token columns instead of ceil-128 blocks, and the
per-slot gate scale moves entirely to the host combine (two column
gathers + scale + add; each token appears in exactly top_k=2 slots).
"""

import numpy as np
import ml_dtypes

import concourse.bass as bass  # noqa: F401  (engine namespace comes via nc)
import concourse.mybir as mybir
import concourse.tile as tile
from concourse import bacc, bass_utils

M, K, N, E, TOPK = 8192, 2048, 1024, 16, 2
NCORES = 8
EPC = E // NCORES  # experts per core
P = 128
KT = K // P   # 16 k-tiles
NT = N // P   # 8 n-tiles

BF16 = mybir.dt.bfloat16
F32 = mybir.dt.float32
NP_BF16 = ml_dtypes.bfloat16

# Set by a driving harness to collect a profile; read back via LAST_RESULT.
TRACE = False
LAST_RESULT = None

_compiled = {}


def _chunks(Ce):
    # 508, not 512: an F=512 chunk makes the moving-operand stride exactly
    # 1KB, and every F=512 region of the trace shows a 432ns PE stall each
    # 10.79us (one lost matmul slot per 50) that F=504/508 regions never
    # show — a bank-conflict beat between the PE's aligned xt reads and
    # concurrent DMA writes. De-aligning removes it; chunk count is
    # unchanged for the capacities this problem produces.
    out = []
    m0 = 0
    while m0 < Ce:
        f = min(508, Ce - m0)
        out.append((m0, f))
        m0 += f
    return out


def _build(caps):
    """caps: per-segment column capacities (seg j of every core holds one
    expert, zero-padded to caps[j]). Sorted pairing on the host means
    caps[0] covers only the small half of the expert-count distribution."""
    CT = sum(caps)
    seg_off = [0]
    for c in caps[:-1]:
        seg_off.append(seg_off[-1] + c)
    nc = bacc.Bacc()
    xt = nc.dram_tensor("xt", [K, CT], BF16, kind="ExternalInput")
    wg = nc.dram_tensor("wg", [EPC, K, N], BF16, kind="ExternalInput")
    wu = nc.dram_tensor("wu", [EPC, K, N], BF16, kind="ExternalInput")
    wd = nc.dram_tensor("wd", [EPC, N, K], BF16, kind="ExternalInput")
    out = nc.dram_tensor("out", [K, CT], BF16, kind="ExternalOutput")

    xt_p = xt.rearrange("(kt p) c -> p kt c", p=P)      # [128, KT, CT]
    out_p = out.rearrange("(kt p) c -> p kt c", p=P)    # [128, KT, CT]

    with tile.TileContext(nc) as tc:
        with (
            tc.tile_pool(name="wpool", bufs=1) as wpool,
            tc.tile_pool(name="xpool", bufs=2) as xpool,
            tc.tile_pool(name="hpool", bufs=3) as hpool,
            tc.tile_pool(name="spool", bufs=8) as spool,
            tc.tile_pool(name="opool", bufs=16) as opool,
            tc.tile_pool(name="psum", bufs=6, space="PSUM") as psum,
            tc.tile_pool(name="psum2", bufs=2, space="PSUM") as psum2,
        ):

            def g1g2(wg_sb, wu_sb, xt_sb, F, kt_outer):
                """Compute H^T for one m-chunk; returns the bf16 ht tile."""
                ht_sb = hpool.tile([P, NT * F], BF16, tag="ht")
                if kt_outer:
                    # All NT accumulation groups open at once so the PE can
                    # consume each wg/xt k-block the moment its DMA lands
                    # (startup: weights are still streaming in from HBM).
                    pgs = [(psum if nt < 6 else psum2).tile([P, F], F32, tag="ps" if nt < 6 else "ps2", name=f"pg{nt}") for nt in range(NT)]
                    # Warm the PE HAM clock gate during the initial weight
                    # stream: ~3.4us of junk matmuls on a memset scratch tile
                    # so the first real groups run at 2.4 GHz, not 1.2.
                    scr = xpool.tile([P, P], BF16, tag="scr", name="scr")
                    nc.vector.memset(scr[:], 0.0)
                    for i in range(32):
                        nc.tensor.matmul(
                            pgs[0][:, :P],
                            scr[:],
                            scr[:],
                            start=(i == 0),
                            stop=(i == 31),
                        )
                    for kt in range(KT):
                        for nt in range(NT):
                            nc.tensor.matmul(
                                pgs[nt][:],
                                wg_sb[:, kt * N + nt * P: kt * N + nt * P + P],
                                xt_sb[:, kt * F: (kt + 1) * F],
                                start=(kt == 0),
                                stop=(kt == KT - 1),
                            )
                    hgs = []
                    for nt in range(NT):
                        hg = spool.tile([P, F], F32, tag="hg")
                        nc.scalar.activation(
                            hg[:], pgs[nt][:], mybir.ActivationFunctionType.Silu
                        )
                        hgs.append(hg)
                    pus = [(psum if nt < 6 else psum2).tile([P, F], F32, tag="ps" if nt < 6 else "ps2", name=f"pu{nt}") for nt in range(NT)]
                    for kt in range(KT):
                        for nt in range(NT):
                            nc.tensor.matmul(
                                pus[nt][:],
                                wu_sb[:, kt * N + nt * P: kt * N + nt * P + P],
                                xt_sb[:, kt * F: (kt + 1) * F],
                                start=(kt == 0),
                                stop=(kt == KT - 1),
                            )
                    for nt in range(NT):
                        nc.vector.tensor_tensor(
                            ht_sb[:, nt * F: (nt + 1) * F],
                            hgs[nt][:],
                            pus[nt][:],
                            mybir.AluOpType.mult,
                        )
                else:
                    for nt in range(NT):
                        pg = psum.tile([P, F], F32, tag="ps")
                        pu = psum.tile([P, F], F32, tag="ps")
                        for kt in range(KT):
                            nc.tensor.matmul(
                                pg[:],
                                wg_sb[:, kt * N + nt * P: kt * N + nt * P + P],
                                xt_sb[:, kt * F: (kt + 1) * F],
                                start=(kt == 0),
                                stop=(kt == KT - 1),
                            )
                        for kt in range(KT):
                            nc.tensor.matmul(
                                pu[:],
                                wu_sb[:, kt * N + nt * P: kt * N + nt * P + P],
                                xt_sb[:, kt * F: (kt + 1) * F],
                                start=(kt == 0),
                                stop=(kt == KT - 1),
                            )
                        hg = spool.tile([P, F], F32, tag="hg")
                        nc.scalar.activation(
                            hg[:], pg[:], mybir.ActivationFunctionType.Silu
                        )
                        nc.vector.tensor_tensor(
                            ht_sb[:, nt * F: (nt + 1) * F],
                            hg[:],
                            pu[:],
                            mybir.AluOpType.mult,
                        )
                return ht_sb

            def g3(wd_sb, ht_sb, F, col0, last=False):
                # Transposed: stationary = Wd [128n, 128k] block, moving =
                # H^T [128n, F]; out rows are K-blocks, cols are tokens.
                # Cost scales with F (no 128-token block quantization).
                # One store DMA per k-block (128KB max — keeps per-queue
                # transfer time ~5us so the final drain never waits long),
                # staged through a 16-deep opool: the copy's buffer-recycle
                # WAR then trails a full chunk (~26us) behind the store,
                # so it never head-of-line blocks the scalar FIFO even when
                # a store queues behind a weight load on the DMA queues.
                # Final chunk: psum->sbuf copies move to the (by then idle)
                # vector engine, stores batch 2 k-blocks and alternate
                # between the sync and scalar HWDGE queues — copies and both
                # issue streams run in parallel, and any copy or issue left
                # over after the last matmul is pure tail latency.
                B = 2 if last else 1
                for kb0 in range(0, KT, B):
                    ot = opool.tile([P, B * F], BF16, tag="ot")
                    for kb in range(kb0, kb0 + B):
                        po = psum2.tile([P, F], F32, tag="ps2")
                        for nt in range(NT):
                            nc.tensor.matmul(
                                po[:],
                                wd_sb[:, nt * K + kb * P: nt * K + kb * P + P],
                                ht_sb[:, nt * F: (nt + 1) * F],
                                start=(nt == 0),
                                stop=(nt == NT - 1),
                            )
                        osl = ot[:, (kb - kb0) * F: (kb - kb0 + 1) * F]
                        if last:
                            nc.vector.tensor_scalar_mul(osl, po[:], 1.0)
                        else:
                            nc.scalar.copy(osl, po[:])
                    eng = nc.scalar if (last and (kb0 // B) % 2) else nc.sync
                    eng.dma_start(
                        out=out_p[:, kb0:kb0 + B, col0:col0 + F],
                        in_=ot[:],
                    )

            # G3 is deferred one chunk (software pipeline): its matmuls fill
            # the PE bubble while the next chunk's silu/mul chain drains, and
            # the next expert's weight DMAs hide under the deferred G3 work.
            pending = None
            for e in range(EPC):
                chunks = _chunks(caps[e])
                wg_sb = wpool.tile([P, KT * N], BF16, tag="wg")
                wu_sb = wpool.tile([P, KT * N], BF16, tag="wu")
                wd_sb = wpool.tile([P, NT * K], BF16, tag="wd")
                wg_d = wg[e].rearrange("(kt p) n -> p kt n", p=P)
                wu_d = wu[e].rearrange("(kt p) n -> p kt n", p=P)
                wd_d = wd[e].rearrange("(nt p) k -> p nt k", p=P)
                # First chunk's activations interleave with wg so the first
                # GEMM group can start as soon as the k-blocks land; wu/wd
                # stream behind it under the shadow of G1/G2 compute.
                F0 = chunks[0][1]
                xt0_sb = xpool.tile([P, KT * F0], BF16, tag="xt")
                if e == 0:
                    for kt in range(KT):
                        if kt == 0:
                            # Split the first weight k-block so the very
                            # first real matmul (kt0, nt0) only waits on
                            # 32KB + the xt k-block, not the full 256KB.
                            nc.sync.dma_start(
                                out=wg_sb[:, 0:P], in_=wg_d[:, 0, 0:P]
                            )
                            nc.sync.dma_start(
                                out=xt0_sb[:, 0:F0],
                                in_=xt_p[:, 0, seg_off[e]:seg_off[e] + F0],
                            )
                            nc.sync.dma_start(
                                out=wg_sb[:, P:N], in_=wg_d[:, 0, P:N]
                            )
                            continue
                        nc.sync.dma_start(
                            out=wg_sb[:, kt * N:(kt + 1) * N],
                            in_=wg_d[:, kt, :],
                        )
                        nc.sync.dma_start(
                            out=xt0_sb[:, kt * F0:(kt + 1) * F0],
                            in_=xt_p[:, kt, seg_off[e]:seg_off[e] + F0],
                        )
                else:
                    for kt in range(KT):
                        nc.sync.dma_start(
                            out=wg_sb[:, kt * N:(kt + 1) * N],
                            in_=wg_d[:, kt, :],
                        )
                        nc.sync.dma_start(
                            out=xt0_sb[:, kt * F0:(kt + 1) * F0],
                            in_=xt_p[:, kt, seg_off[e]:seg_off[e] + F0],
                        )
                for kt in range(KT):
                    nc.sync.dma_start(
                        out=wu_sb[:, kt * N:(kt + 1) * N], in_=wu_d[:, kt, :]
                    )
                for nt in range(NT):
                    nc.sync.dma_start(
                        out=wd_sb[:, nt * K:(nt + 1) * K], in_=wd_d[:, nt, :]
                    )
                if pending is not None:
                    # Flush after this expert's weight-DMA emission (so the
                    # transfers start under this G3's compute cover) but
                    # before its first GEMM group: the PE is in-order, and
                    # this G3's work is ready now while the new expert's
                    # matmuls would head-of-line block on the weight DMAs.
                    g3(*pending)
                    pending = None
                for ci, (m0, F) in enumerate(chunks):
                    col0 = seg_off[e] + m0
                    if ci == 0:
                        xt_sb = xt0_sb
                    else:
                        # Batch 2 k-tiles per load descriptor: halves the
                        # serial sync-engine issue cost without creating
                        # 512KB single-queue transfers that output stores
                        # would queue behind.
                        xt_sb = xpool.tile([P, KT * F], BF16, tag="xt")
                        for kt in range(0, KT, 2):
                            nc.sync.dma_start(
                                out=xt_sb[:, kt * F:(kt + 2) * F],
                                in_=xt_p[:, kt:kt + 2, col0:col0 + F],
                            )
                    ht_sb = g1g2(wg_sb, wu_sb, xt_sb, F, kt_outer=(e == 0 and ci == 0))
                    if pending is not None:
                        g3(*pending)
                    pending = (wd_sb, ht_sb, F, col0)
            if pending is not None:
                g3(*pending, last=True)
    nc.compile()
    return nc


def _get(caps):
    if caps not in _compiled:
        _compiled[caps] = _build(caps)
    return _compiled[caps]


def _r2(x):
    return int(max(-(-int(x) // 2) * 2, 8))


def kernel(flat_h, flat_idx, flat_gate, gate_weight, up_weight, down_weight):
    global LAST_RESULT
    eid = np.asarray(flat_idx).reshape(-1).astype(np.int64)
    gvals = np.asarray(flat_gate).reshape(-1).astype(np.float32)
    perm = np.argsort(eid, kind="stable")
    cnt = np.bincount(eid, minlength=E)
    offs = np.concatenate([[0], np.cumsum(cnt)])
    # Sorted pairing: core c gets the c-th smallest (segment 0) and c-th
    # largest (segment 1) expert, so segment 0's capacity only covers the
    # small half of the count distribution instead of the global max.
    order = np.argsort(-cnt, kind="stable")
    Cb = _r2(cnt[order[0]])
    Cs = _r2(cnt[order[NCORES]])
    caps = (Cs, Cb)
    CT = Cs + Cb
    seg_off = [0, Cs]
    nc = _get(caps)

    Xb = np.asarray(flat_h).astype(NP_BF16)
    wgT = gate_weight.transpose(0, 2, 1).astype(NP_BF16)  # (E, K, N)
    wuT = up_weight.transpose(0, 2, 1).astype(NP_BF16)    # (E, K, N)
    wdT = down_weight.transpose(0, 2, 1).astype(NP_BF16)  # (E, N, K)
    wgT = np.ascontiguousarray(wgT)
    wuT = np.ascontiguousarray(wuT)
    wdT = np.ascontiguousarray(wdT)

    colmap = np.zeros(M * TOPK, dtype=np.int64)
    in_maps = []
    for c in range(NCORES):
        xt = np.zeros((K, CT), dtype=NP_BF16)
        exs = [int(order[2 * NCORES - 1 - c]), int(order[c])]
        for j, e in enumerate(exs):
            rows = perm[offs[e]:offs[e + 1]]
            n_e = len(rows)
            xt[:, seg_off[j]: seg_off[j] + n_e] = Xb[rows // TOPK].T
            colmap[rows] = c * CT + seg_off[j] + np.arange(n_e)
        in_maps.append(
            {
                "xt": xt,
                "wg": np.ascontiguousarray(np.stack([wgT[e] for e in exs])),
                "wu": np.ascontiguousarray(np.stack([wuT[e] for e in exs])),
                "wd": np.ascontiguousarray(np.stack([wdT[e] for e in exs])),
            }
        )

    res = bass_utils.run_bass_kernel_spmd(
        nc, in_maps, core_ids=list(range(NCORES)), trace=TRACE
    )
    LAST_RESULT = res
    # Per-core outputs are [K, CT] bf16, token-major on the free axis; the
    # combine is two column gathers + the per-slot gate scale + add.
    Y = np.concatenate(
        [np.asarray(res.results[c]["out"]) for c in range(NCORES)], axis=1
    )
    out = (
        Y[:, colmap[0::2]].astype(np.float32) * gvals[0::2]
        + Y[:, colmap[1::2]].astype(np.float32) * gvals[1::2]
    ).T
    return np.ascontiguousarray(out, dtype=np.float32)


# revision 23
# speedup vs baseline: 1.0052x; 1.0052x over previous
"""Grouped SwiGLU MoE (M=8192, K=2048, N=1024, E=16, top-2) on 8 TRN2 cores.

Strategy: expert-parallel. Host sorts the M*top_k expanded token slots by
expert, gathers+transposes the activations per core (core c owns the c-th
smallest expert in segment 0 and the c-th largest in segment 1; segments
are zero-padded to static capacities rounded to 8, not 128), and
pre-transposes the three weight tensors to bf16. Each core runs a dense
per-expert GEMM chain:

    H^T[n, m] = silu(Wg^T-contract) * (Wu^T-contract)     (PSUM f32, bf16 out)
    outT[k, m] = Wd^T-stationary-contract over n          (bf16 out, [K, CT])

G3 is transposed (stationary = Wd block, moving = H^T), so its PE cost
scales with actual token columns instead of ceil-128 blocks, and the
per-slot gate scale moves entirely to the host combine (two column
gathers + scale + add; each token appears in exactly top_k=2 slots).
"""

import numpy as np
import ml_dtypes

import concourse.bass as bass  # noqa: F401  (engine namespace comes via nc)
import concourse.mybir as mybir
import concourse.tile as tile
from concourse import bacc, bass_utils

M, K, N, E, TOPK = 8192, 2048, 1024, 16, 2
NCORES = 8
EPC = E // NCORES  # experts per core
P = 128
KT = K // P   # 16 k-tiles
NT = N // P   # 8 n-tiles

BF16 = mybir.dt.bfloat16
F32 = mybir.dt.float32
NP_BF16 = ml_dtypes.bfloat16

# Set by a driving harness to collect a profile; read back via LAST_RESULT.
TRACE = False
LAST_RESULT = None

_compiled = {}


def _chunks(Ce):
    # 508, not 512: an F=512 chunk makes the moving-operand stride exactly
    # 1KB, and every F=512 region of the trace shows a 432ns PE stall each
    # 10.79us (one lost matmul slot per 50) that F=504/508 regions never
    # show — a bank-conflict beat between the PE's aligned xt reads and
    # concurrent DMA writes. De-aligning removes it; chunk count is
    # unchanged for the capacities this problem produces.
    out = []
    m0 = 0
    while m0 < Ce:
        f = min(508, Ce - m0)
        out.append((m0, f))
        m0 += f
    return out


def _build(caps):
    """caps: per-segment column capacities (seg j of every core holds one
    expert, zero-padded to caps[j]). Sorted pairing on the host means
    caps[0] covers only the small half of the expert-count distribution."""
    CT = sum(caps)
    seg_off = [0]
    for c in caps[:-1]:
        seg_off.append(seg_off[-1] + c)
    nc = bacc.Bacc()
    xt = nc.dram_tensor("xt", [K, CT], BF16, kind="ExternalInput")
    wg = nc.dram_tensor("wg", [EPC, K, N], BF16, kind="ExternalInput")
    wu = nc.dram_tensor("wu", [EPC, K, N], BF16, kind="ExternalInput")
    wd = nc.dram_tensor("wd", [EPC, N, K], BF16, kind="ExternalInput")
    out = nc.dram_tensor("out", [K, CT], BF16, kind="ExternalOutput")

    xt_p = xt.rearrange("(kt p) c -> p kt c", p=P)      # [128, KT, CT]
    out_p = out.rearrange("(kt p) c -> p kt c", p=P)    # [128, KT, CT]

    with tile.TileContext(nc) as tc:
        with (
            tc.tile_pool(name="wpool", bufs=1) as wpool,
            tc.tile_pool(name="xpool", bufs=2) as xpool,
            tc.tile_pool(name="hpool", bufs=3) as hpool,
            tc.tile_pool(name="spool", bufs=8) as spool,
            tc.tile_pool(name="opool", bufs=16) as opool,
            tc.tile_pool(name="psum", bufs=6, space="PSUM") as psum,
            tc.tile_pool(name="psum2", bufs=2, space="PSUM") as psum2,
        ):

            def g1g2(wg_sb, wu_sb, xt_sb, F, kt_outer):
                """Compute H^T for one m-chunk; returns the bf16 ht tile."""
                ht_sb = hpool.tile([P, NT * F], BF16, tag="ht")
                if kt_outer:
                    # All NT accumulation groups open at once so the PE can
                    # consume each wg/xt k-block the moment its DMA lands
                    # (startup: weights are still streaming in from HBM).
                    pgs = [(psum if nt < 6 else psum2).tile([P, F], F32, tag="ps" if nt < 6 else "ps2", name=f"pg{nt}") for nt in range(NT)]
                    # Warm the PE HAM clock gate during the initial weight
                    # stream: ~3.4us of junk matmuls on a memset scratch tile
                    # so the first real groups run at 2.4 GHz, not 1.2.
                    scr = xpool.tile([P, P], BF16, tag="scr", name="scr")
                    nc.vector.memset(scr[:], 0.0)
                    for i in range(32):
                        nc.tensor.matmul(
                            pgs[0][:, :P],
                            scr[:],
                            scr[:],
                            start=(i == 0),
                            stop=(i == 31),
                        )
                    for kt in range(KT):
                        for nt in range(NT):
                            nc.tensor.matmul(
                                pgs[nt][:],
                                wg_sb[:, kt * N + nt * P: kt * N + nt * P + P],
                                xt_sb[:, kt * F: (kt + 1) * F],
                                start=(kt == 0),
                                stop=(kt == KT - 1),
                            )
                    hgs = []
                    for nt in range(NT):
                        hg = spool.tile([P, F], F32, tag="hg")
                        nc.scalar.activation(
                            hg[:], pgs[nt][:], mybir.ActivationFunctionType.Silu
                        )
                        hgs.append(hg)
                    pus = [(psum if nt < 6 else psum2).tile([P, F], F32, tag="ps" if nt < 6 else "ps2", name=f"pu{nt}") for nt in range(NT)]
                    for kt in range(KT):
                        for nt in range(NT):
                            nc.tensor.matmul(
                                pus[nt][:],
                                wu_sb[:, kt * N + nt * P: kt * N + nt * P + P],
                                xt_sb[:, kt * F: (kt + 1) * F],
                                start=(kt == 0),
                                stop=(kt == KT - 1),
                            )
                    for nt in range(NT):
                        nc.vector.tensor_tensor(
                            ht_sb[:, nt * F: (nt + 1) * F],
                            hgs[nt][:],
                            pus[nt][:],
                            mybir.AluOpType.mult,
                        )
                else:
                    for nt in range(NT):
                        pg = psum.tile([P, F], F32, tag="ps")
                        pu = psum.tile([P, F], F32, tag="ps")
                        for kt in range(KT):
                            nc.tensor.matmul(
                                pg[:],
                                wg_sb[:, kt * N + nt * P: kt * N + nt * P + P],
                                xt_sb[:, kt * F: (kt + 1) * F],
                                start=(kt == 0),
                                stop=(kt == KT - 1),
                            )
                        for kt in range(KT):
                            nc.tensor.matmul(
                                pu[:],
                                wu_sb[:, kt * N + nt * P: kt * N + nt * P + P],
                                xt_sb[:, kt * F: (kt + 1) * F],
                                start=(kt == 0),
                                stop=(kt == KT - 1),
                            )
                        hg = spool.tile([P, F], F32, tag="hg")
                        nc.scalar.activation(
                            hg[:], pg[:], mybir.ActivationFunctionType.Silu
                        )
                        nc.vector.tensor_tensor(
                            ht_sb[:, nt * F: (nt + 1) * F],
                            hg[:],
                            pu[:],
                            mybir.AluOpType.mult,
                        )
                return ht_sb

            def g3(wd_sb, ht_sb, F, col0, last=False):
                # Transposed: stationary = Wd [128n, 128k] block, moving =
                # H^T [128n, F]; out rows are K-blocks, cols are tokens.
                # Cost scales with F (no 128-token block quantization).
                # One store DMA per k-block (128KB max — keeps per-queue
                # transfer time ~5us so the final drain never waits long),
                # staged through a 16-deep opool: the copy's buffer-recycle
                # WAR then trails a full chunk (~26us) behind the store,
                # so it never head-of-line blocks the scalar FIFO even when
                # a store queues behind a weight load on the DMA queues.
                # Final chunk: psum->sbuf copies move to the (by then idle)
                # vector engine, stores batch 2 k-blocks and alternate
                # between the sync and scalar HWDGE queues — copies and both
                # issue streams run in parallel, and any copy or issue left
                # over after the last matmul is pure tail latency.
                B = 2 if last else 1
                for kb0 in range(0, KT, B):
                    ot = opool.tile([P, B * F], BF16, tag="ot")
                    for kb in range(kb0, kb0 + B):
                        # Final chunk: use the 6-bank pool (G1/G2 are done)
                        # — the 2-bank pool's recycle is tighter than the
                        # small-F group time and stalls the PE.
                        pp = psum if last else psum2
                        po = pp.tile([P, F], F32, tag="ps" if last else "ps2")
                        for nt in range(NT):
                            nc.tensor.matmul(
                                po[:],
                                wd_sb[:, nt * K + kb * P: nt * K + kb * P + P],
                                ht_sb[:, nt * F: (nt + 1) * F],
                                start=(nt == 0),
                                stop=(nt == NT - 1),
                            )
                        osl = ot[:, (kb - kb0) * F: (kb - kb0 + 1) * F]
                        if last:
                            nc.vector.tensor_scalar_mul(osl, po[:], 1.0)
                        else:
                            nc.scalar.copy(osl, po[:])
                    eng = nc.scalar if (last and (kb0 // B) % 2) else nc.sync
                    eng.dma_start(
                        out=out_p[:, kb0:kb0 + B, col0:col0 + F],
                        in_=ot[:],
                    )

            # G3 is deferred one chunk (software pipeline): its matmuls fill
            # the PE bubble while the next chunk's silu/mul chain drains, and
            # the next expert's weight DMAs hide under the deferred G3 work.
            pending = None
            for e in range(EPC):
                chunks = _chunks(caps[e])
                wg_sb = wpool.tile([P, KT * N], BF16, tag="wg")
                wu_sb = wpool.tile([P, KT * N], BF16, tag="wu")
                wd_sb = wpool.tile([P, NT * K], BF16, tag="wd")
                wg_d = wg[e].rearrange("(kt p) n -> p kt n", p=P)
                wu_d = wu[e].rearrange("(kt p) n -> p kt n", p=P)
                wd_d = wd[e].rearrange("(nt p) k -> p nt k", p=P)
                # First chunk's activations interleave with wg so the first
                # GEMM group can start as soon as the k-blocks land; wu/wd
                # stream behind it under the shadow of G1/G2 compute.
                F0 = chunks[0][1]
                xt0_sb = xpool.tile([P, KT * F0], BF16, tag="xt")
                if e == 0:
                    for kt in range(KT):
                        if kt == 0:
                            # Split the first weight k-block so the very
                            # first real matmul (kt0, nt0) only waits on
                            # 32KB + the xt k-block, not the full 256KB.
                            nc.sync.dma_start(
                                out=wg_sb[:, 0:P], in_=wg_d[:, 0, 0:P]
                            )
                            nc.sync.dma_start(
                                out=xt0_sb[:, 0:F0],
                                in_=xt_p[:, 0, seg_off[e]:seg_off[e] + F0],
                            )
                            nc.sync.dma_start(
                                out=wg_sb[:, P:N], in_=wg_d[:, 0, P:N]
                            )
                            continue
                        nc.sync.dma_start(
                            out=wg_sb[:, kt * N:(kt + 1) * N],
                            in_=wg_d[:, kt, :],
                        )
                        nc.sync.dma_start(
                            out=xt0_sb[:, kt * F0:(kt + 1) * F0],
                            in_=xt_p[:, kt, seg_off[e]:seg_off[e] + F0],
                        )
                else:
                    for kt in range(KT):
                        nc.sync.dma_start(
                            out=wg_sb[:, kt * N:(kt + 1) * N],
                            in_=wg_d[:, kt, :],
                        )
                        nc.sync.dma_start(
                            out=xt0_sb[:, kt * F0:(kt + 1) * F0],
                            in_=xt_p[:, kt, seg_off[e]:seg_off[e] + F0],
                        )
                for kt in range(KT):
                    nc.sync.dma_start(
                        out=wu_sb[:, kt * N:(kt + 1) * N], in_=wu_d[:, kt, :]
                    )
                for nt in range(NT):
                    nc.sync.dma_start(
                        out=wd_sb[:, nt * K:(nt + 1) * K], in_=wd_d[:, nt, :]
                    )
                if pending is not None:
                    # Flush after this expert's weight-DMA emission (so the
                    # transfers start under this G3's compute cover) but
                    # before its first GEMM group: the PE is in-order, and
                    # this G3's work is ready now while the new expert's
                    # matmuls would head-of-line block on the weight DMAs.
                    g3(*pending)
                    pending = None
                for ci, (m0, F) in enumerate(chunks):
                    col0 = seg_off[e] + m0
                    if ci == 0:
                        xt_sb = xt0_sb
                    else:
                        # Batch 2 k-tiles per load descriptor: halves the
                        # serial sync-engine issue cost without creating
                        # 512KB single-queue transfers that output stores
                        # would queue behind.
                        xt_sb = xpool.tile([P, KT * F], BF16, tag="xt")
                        for kt in range(0, KT, 2):
                            nc.sync.dma_start(
                                out=xt_sb[:, kt * F:(kt + 2) * F],
                                in_=xt_p[:, kt:kt + 2, col0:col0 + F],
                            )
                    ht_sb = g1g2(wg_sb, wu_sb, xt_sb, F, kt_outer=(e == 0 and ci == 0))
                    if pending is not None:
                        g3(*pending)
                    pending = (wd_sb, ht_sb, F, col0)
            if pending is not None:
                g3(*pending, last=True)
    nc.compile()
    return nc


def _get(caps):
    if caps not in _compiled:
        _compiled[caps] = _build(caps)
    return _compiled[caps]


def _r2(x):
    return int(max(-(-int(x) // 2) * 2, 8))


def kernel(flat_h, flat_idx, flat_gate, gate_weight, up_weight, down_weight):
    global LAST_RESULT
    eid = np.asarray(flat_idx).reshape(-1).astype(np.int64)
    gvals = np.asarray(flat_gate).reshape(-1).astype(np.float32)
    perm = np.argsort(eid, kind="stable")
    cnt = np.bincount(eid, minlength=E)
    offs = np.concatenate([[0], np.cumsum(cnt)])
    # Sorted pairing: core c gets the c-th smallest (segment 0) and c-th
    # largest (segment 1) expert, so segment 0's capacity only covers the
    # small half of the count distribution instead of the global max.
    order = np.argsort(-cnt, kind="stable")
    Cb = _r2(cnt[order[0]])
    Cs = _r2(cnt[order[NCORES]])
    caps = (Cs, Cb)
    CT = Cs + Cb
    seg_off = [0, Cs]
    nc = _get(caps)

    Xb = np.asarray(flat_h).astype(NP_BF16)
    wgT = gate_weight.transpose(0, 2, 1).astype(NP_BF16)  # (E, K, N)
    wuT = up_weight.transpose(0, 2, 1).astype(NP_BF16)    # (E, K, N)
    wdT = down_weight.transpose(0, 2, 1).astype(NP_BF16)  # (E, N, K)
    wgT = np.ascontiguousarray(wgT)
    wuT = np.ascontiguousarray(wuT)
    wdT = np.ascontiguousarray(wdT)

    colmap = np.zeros(M * TOPK, dtype=np.int64)
    in_maps = []
    for c in range(NCORES):
        xt = np.zeros((K, CT), dtype=NP_BF16)
        exs = [int(order[2 * NCORES - 1 - c]), int(order[c])]
        for j, e in enumerate(exs):
            rows = perm[offs[e]:offs[e + 1]]
            n_e = len(rows)
            xt[:, seg_off[j]: seg_off[j] + n_e] = Xb[rows // TOPK].T
            colmap[rows] = c * CT + seg_off[j] + np.arange(n_e)
        in_maps.append(
            {
                "xt": xt,
                "wg": np.ascontiguousarray(np.stack([wgT[e] for e in exs])),
                "wu": np.ascontiguousarray(np.stack([wuT[e] for e in exs])),
                "wd": np.ascontiguousarray(np.stack([wdT[e] for e in exs])),
            }
        )

    res = bass_utils.run_bass_kernel_spmd(
        nc, in_maps, core_ids=list(range(NCORES)), trace=TRACE
    )
    LAST_RESULT = res
    # Per-core outputs are [K, CT] bf16, token-major on the free axis; the
    # combine is two column gathers + the per-slot gate scale + add.
    Y = np.concatenate(
        [np.asarray(res.results[c]["out"]) for c in range(NCORES)], axis=1
    )
    out = (
        Y[:, colmap[0::2]].astype(np.float32) * gvals[0::2]
        + Y[:, colmap[1::2]].astype(np.float32) * gvals[1::2]
    ).T
    return np.ascontiguousarray(out, dtype=np.float32)
